# revision 1
# baseline (speedup 1.0000x reference)
"""Trainium2 Bass kernel for nn_BLayer_63780264346268 (topk_masking).

Math (per output unit o of 512):
  idx = top6(mask[o])                                  (6 of 1024 input features)
  h1 = relu(x[:, idx] @ W1[o, idx, :])                 (B,6)@(6,32)
  h2 = relu(h1 @ W2[o]); h3 = relu(h2 @ W3[o])         (B,32)@(32,32)
  y  = sigmoid(h3 @ W4[o]); q = (y>=.5)*2-1  == sign(h3 @ W4[o]) (as +/-1)

Distribution: shard the 512 output units across 8 cores (64 each), full
inputs replicated in DRAM (only gathered rows are read). Host does layout
prep + final concat; top-k, gathers and all math run on device.

Device flow (per core, 64 units indexed o = m*16 + i*4 + j; partition
p = 32m + 8j + j6 within strip i):
  - top-8 values+indices per unit via DVE max/max_index on permuted mask.
  - per strip i an index column [128,1] feeds an indirect-DMA row gather
    (one row per partition):
      stageX[32m+8j+j6, 256i+b] = xT[idx[o,j6], b]   (j6>=6 -> zero row)
      stageW[32m+8j+j6, 32i+h]  = W1flat[o*1024+idx[o,j6], h]
  - block-diag bd[32m+8j+j6, 128i+32j'+h] = delta(j,j')*stageW (16 small
    SBUF->SBUF DMAs split over both HWDGE queues), so L1 is one
    (K=32, M=128, N=256) matmul per (m,i) on row-quadrant m:
    out[32j+h, b] = h1[(m,i,j)][h, b].
  - L2/L3: one (K=128, M=128, N=256) matmul per (m,i) against host-built
    block-diagonal weights (W3 columns pre-scaled by |W4|); row layout
    (j, hid) is preserved layer to layer.
  - L4: per (m,i) one (K=128, M=4, N=256) matmul against sign(W4) block
    columns, then Sign activation = the binarize.
  - PSUM tiles pair two matmuls per bank [128, 512]; relu evacuations
    alternate between Scalar and Vector engines.
"""

import numpy as np

OUT, IN, HID, B = 512, 1024, 32, 256
NCORES = 8
OSH = OUT // NCORES  # 64 output units per core

_CACHE = {}


def _perm():
    # mask-tile row p = i*16 + m*4 + j  <->  unit o = m*16 + i*4 + j
    p = np.arange(64)
    i, r = p // 16, p % 16
    m, j = r // 4, r % 4
    return (m * 16 + i * 4 + j).astype(np.int64)


def _build_program():
    import concourse.bacc as bacc
    import concourse.bass as bass
    import concourse.mybir as mybir
    import concourse.tile as tile

    f32 = mybir.dt.float32
    u32 = mybir.dt.uint32
    RELU = mybir.ActivationFunctionType.Relu
    SIGN = mybir.ActivationFunctionType.Sign

    nc = bacc.Bacc(None, target_bir_lowering=False, debug=False)

    xT = nc.dram_tensor("xT", [IN + 1, B], f32, kind="ExternalInput")
    maskS = nc.dram_tensor("maskS", [OSH, IN], f32, kind="ExternalInput")
    oconst = nc.dram_tensor("oconst", [OSH, 1], u32, kind="ExternalInput")
    w1f = nc.dram_tensor("w1f", [OSH * IN, HID], f32, kind="ExternalInput")
    w2bd_d = nc.dram_tensor("w2bd", [128, 2048], f32, kind="ExternalInput")
    w3bd_d = nc.dram_tensor("w3bd", [128, 2048], f32, kind="ExternalInput")
    w4sgn = nc.dram_tensor("w4sgn", [128, 64], f32, kind="ExternalInput")
    outS = nc.dram_tensor("outS", [OSH, B], f32, kind="ExternalOutput")

    with tile.TileContext(nc) as tc:
        with (
            tc.tile_pool(name="const", bufs=1) as cpool,
            tc.tile_pool(name="ps", bufs=8, space="PSUM") as pspool,
        ):
            # sync-queue loads (small, needed early)
            mask_t = cpool.tile([OSH, IN], f32)
            nc.sync.dma_start(mask_t[:], maskS[:])
            oconst_t = cpool.tile([OSH, 1], u32)
            nc.sync.dma_start(oconst_t[:], oconst[:])
            w4t = cpool.tile([128, 64], f32)
            nc.sync.dma_start(w4t[:], w4sgn[:])

            # --- top-8 values + indices per unit (6 real, 2 zeroed) ---
            mx8 = cpool.tile([OSH, 8], f32)
            idx8 = cpool.tile([OSH, 8], u32)
            nc.vector.max(out=mx8[:], in_=mask_t[:])
            nc.vector.max_index(out=idx8[:], in_max=mx8[:], in_values=mask_t[:])
            gw = cpool.tile([OSH, 8], u32)  # o*1024 + idx : rows of w1f
            nc.vector.tensor_tensor(
                out=gw[:],
                in0=idx8[:],
                in1=oconst_t[:].to_broadcast([OSH, 8]),
                op=mybir.AluOpType.add,
            )
            gx = cpool.tile([OSH, 8], u32)  # idx, cols 6..7 -> zero row of xT
            nc.vector.tensor_copy(gx[:, 0:6], idx8[:, 0:6])
            nc.vector.memset(gx[:, 6:8], IN)

            # one index per partition for strip i: partition 8*(row-16i)+j6
            idxw = [
                cpool.tile([128, 1], u32, name=f"idxw_{i}", tag=f"idxw_{i}")
                for i in range(4)
            ]
            idxx = [
                cpool.tile([128, 1], u32, name=f"idxx_{i}", tag=f"idxx_{i}")
                for i in range(4)
            ]
            for i in range(4):
                weng = nc.sync if i % 2 == 0 else nc.scalar
                xeng = nc.scalar if i % 2 == 0 else nc.sync
                weng.dma_start(out=idxw[i][:], in_=gw[16 * i : 16 * i + 16, 0:8])
                xeng.dma_start(out=idxx[i][:], in_=gx[16 * i : 16 * i + 16, 0:8])

            # big weight loads ride the scalar HWDGE queue during the
            # topk/gather window
            w2bd = cpool.tile([128, 2048], f32)
            nc.scalar.dma_start(w2bd[:], w2bd_d[:])
            w3bd = cpool.tile([128, 2048], f32)
            nc.scalar.dma_start(w3bd[:], w3bd_d[:])

            # --- row gathers (W strips first; X overlaps bd/L1) ---
            stageW = cpool.tile([128, 128], f32)
            stageX = cpool.tile([128, 1024], f32)
            for i in range(4):
                nc.gpsimd.indirect_dma_start(
                    out=stageW[:, 32 * i : 32 * i + 32],
                    out_offset=None,
                    in_=w1f[:, :],
                    in_offset=bass.IndirectOffsetOnAxis(ap=idxw[i][:, 0:1], axis=0),
                )
            for i in range(4):
                nc.gpsimd.indirect_dma_start(
                    out=stageX[:, B * i : B * i + B],
                    out_offset=None,
                    in_=xT[:, :],
                    in_offset=bass.IndirectOffsetOnAxis(ap=idxx[i][:, 0:1], axis=0),
                )

            # --- block-diagonal W1: bd[32m+8j+j6, 128i+32j+h] ---
            bd = cpool.tile([128, 512], f32)
            nc.vector.memset(bd[:], 0.0)
            bd4 = bd.rearrange("p (i jj h) -> p i jj h", jj=4, h=HID)
            for m in range(4):
                for j in range(4):
                    r0 = 32 * m + 8 * j
                    eng = nc.sync if (m * 4 + j) % 2 == 0 else nc.scalar
                    eng.dma_start(
                        out=bd4[r0 : r0 + 8, :, j, :],
                        in_=stageW[r0 : r0 + 8, :].rearrange(
                            "p (i h) -> p i h", h=HID
                        ),
                    )

            def evac(dst, src, k):
                # alternate relu evacuation between Scalar and Vector
                if k % 2 == 0:
                    nc.scalar.activation(out=dst, in_=src, func=RELU)
                else:
                    nc.vector.tensor_scalar_max(dst, src, 0.0)

            # --- L1: (K=32, M=128, N=256) per (m, i), row-tiled by m ---
            h1s = cpool.tile([128, 4096], f32)
            nk = 0
            for m in range(4):
                for i2 in range(2):
                    ps1 = pspool.tile(
                        [128, 512], f32, tag="ps", name=f"ps1_{m}_{i2}"
                    )
                    for ih in range(2):
                        i = 2 * i2 + ih
                        nc.tensor.matmul(
                            out=ps1[:, 256 * ih : 256 * ih + 256],
                            lhsT=bd[32 * m : 32 * m + 32, 128 * i : 128 * i + 128],
                            rhs=stageX[32 * m : 32 * m + 32, B * i : B * i + B],
                            start=True,
                            stop=True,
                            tile_position=(32 * m, 0),
                        )
                    q0 = 4 * m + 2 * i2
                    evac(h1s[:, B * q0 : B * q0 + 512], ps1[:, :], nk)
                    nk += 1

            # --- L2/L3: (K=128, M=128, N=256) per (m, i), block-diag ---
            h2s = cpool.tile([128, 4096], f32)
            h3s = cpool.tile([128, 4096], f32)
            ys = cpool.tile([128, 1024], f32)
            for m in range(4):
                for i2 in range(2):
                    ps2 = pspool.tile(
                        [128, 512], f32, tag="ps", name=f"ps2_{m}_{i2}"
                    )
                    for ih in range(2):
                        q = 4 * m + 2 * i2 + ih
                        nc.tensor.matmul(
                            out=ps2[:, 256 * ih : 256 * ih + 256],
                            lhsT=w2bd[:, 128 * q : 128 * q + 128],
                            rhs=h1s[:, B * q : B * q + B],
                            start=True,
                            stop=True,
                            tile_position=(0, 0),
                        )
                    q0 = 4 * m + 2 * i2
                    evac(h2s[:, B * q0 : B * q0 + 512], ps2[:, :], nk)
                    nk += 1
            for m in range(4):
                for i2 in range(2):
                    ps3 = pspool.tile(
                        [128, 512], f32, tag="ps", name=f"ps3_{m}_{i2}"
                    )
                    for ih in range(2):
                        q = 4 * m + 2 * i2 + ih
                        nc.tensor.matmul(
                            out=ps3[:, 256 * ih : 256 * ih + 256],
                            lhsT=w3bd[:, 128 * q : 128 * q + 128],
                            rhs=h2s[:, B * q : B * q + B],
                            start=True,
                            stop=True,
                            tile_position=(0, 0),
                        )
                    q0 = 4 * m + 2 * i2
                    evac(h3s[:, B * q0 : B * q0 + 512], ps3[:, :], nk)
                    nk += 1

            # --- L4 + binarize ---
            # psum bank per i, column quadrant per m: psy_i[32m:32m+4] = y(m,i,:)
            psy = [
                pspool.tile([128, 512], f32, tag="ps", name=f"psy_{i}")
                for i in range(4)
            ]
            for m in range(4):
                for i in range(4):
                    t = m * 4 + i
                    nc.tensor.matmul(
                        out=psy[i][32 * m : 32 * m + 4, 0:256],
                        lhsT=w4t[:, 4 * t : 4 * t + 4],
                        rhs=h3s[:, B * t : B * t + B],
                        start=True,
                        stop=True,
                        tile_position=(0, 32 * m),
                    )
            for i in range(4):
                nc.scalar.activation(
                    out=ys[:, B * i : B * i + B], in_=psy[i][:, 0:256], func=SIGN
                )
            # outS[16m+4i+j, b] = ys[32m+j, 256i+b], one DMA per m
            for m in range(4):
                eng = nc.sync if m % 2 == 0 else nc.scalar
                eng.dma_start(
                    out=outS[16 * m : 16 * m + 16, :].rearrange(
                        "(i j) b -> j i b", j=4
                    ),
                    in_=ys[32 * m : 32 * m + 4, :].rearrange(
                        "j (i b) -> j i b", b=B
                    ),
                )

    nc.compile()
    return nc


def _prep_core(c, inputs, mask, W1, W2, W3, W4, perm):
    sl = slice(c * OSH, (c + 1) * OSH)
    mask_c = np.ascontiguousarray(mask[sl])
    W1c, W2c, W3c, W4c = W1[sl], W2[sl], W3[sl], W4[sl]

    maskS = np.ascontiguousarray(mask_c[perm])
    oconst = (perm.astype(np.uint32) * np.uint32(IN))[:, None]
    w1f = np.ascontiguousarray(W1c.reshape(OSH * IN, HID))

    # block-diag L2/L3 weights: col block q=4m+i holds lhsT for (m,i):
    #   w2bd[32j+h, 128q + 32j+k] = W2c[o(m,i,j), h, k]
    #   w3bd[32j+k, 128q + 32j+l] = W3c[o,k,l] * |W4c[o,l]|
    w4v = W4c[:, :, 0]  # [64, 32]
    w3p = W3c * np.abs(w4v)[:, None, :]
    w2bd = np.zeros((128, 2048), np.float32)
    w3bd = np.zeros((128, 2048), np.float32)
    for o in range(OSH):
        m, i, j = o // 16, (o % 16) // 4, o % 4
        q = 4 * m + i
        w2bd[32 * j : 32 * j + 32, 128 * q + 32 * j : 128 * q + 32 * j + 32] = W2c[o]
        w3bd[32 * j : 32 * j + 32, 128 * q + 32 * j : 128 * q + 32 * j + 32] = w3p[o]
    # w4sgn[32j+l, (m*4+i)*4+j] = sign(W4c[m*16+i*4+j, l])
    sgn = np.sign(w4v).astype(np.float32)
    w4sgn = np.zeros((128, 64), np.float32)
    for o in range(OSH):
        m, i, j = o // 16, (o % 16) // 4, o % 4
        w4sgn[32 * j : 32 * j + 32, (m * 4 + i) * 4 + j] = sgn[o]

    return {
        "maskS": maskS.astype(np.float32),
        "oconst": oconst,
        "w1f": w1f.astype(np.float32),
        "w2bd": w2bd,
        "w3bd": w3bd,
        "w4sgn": w4sgn,
    }


def kernel(inputs, mask, W1, W2, W3, W4, _run_kwargs=None):
    from concourse.bass_utils import run_bass_kernel_spmd

    inputs = np.asarray(inputs, np.float32)
    mask = np.asarray(mask, np.float32)
    W1 = np.asarray(W1, np.float32)
    W2 = np.asarray(W2, np.float32)
    W3 = np.asarray(W3, np.float32)
    W4 = np.asarray(W4, np.float32)

    if "nc" not in _CACHE:
        _CACHE["nc"] = _build_program()
    nc = _CACHE["nc"]

    perm = _perm()
    xT = np.zeros((IN + 1, B), np.float32)
    xT[:IN] = inputs.T
    in_maps = []
    for c in range(NCORES):
        m = _prep_core(c, inputs, mask, W1, W2, W3, W4, perm)
        m["xT"] = xT
        in_maps.append(m)

    kw = dict(_run_kwargs or {})
    res = run_bass_kernel_spmd(nc, in_maps, core_ids=list(range(NCORES)), **kw)
    out = np.concatenate([r["outS"].T for r in res.results], axis=1)
    if _run_kwargs is not None:
        _CACHE["last_result"] = res
    return out.astype(np.float32)



# revision 12
# speedup vs baseline: 1.3231x; 1.3231x over previous
"""Trainium2 Bass kernel for nn_BLayer_63780264346268 (topk_masking).

Math (per output unit o of 512):
  idx = top6(mask[o])                                  (6 of 1024 input features)
  h1 = relu(x[:, idx] @ W1[o, idx, :])                 (B,6)@(6,32)
  h2 = relu(h1 @ W2[o]); h3 = relu(h2 @ W3[o])         (B,32)@(32,32)
  y  = sigmoid(h3 @ W4[o]); q = (y>=.5)*2-1  == sign(h3 @ W4[o]) (as +/-1)

Distribution: 512 output units sharded across 8 cores (64 each). Top-k,
gathers and all math run on device; host does layout prep + final concat.

Per-core layout (64 units; o = 16i + 4m + j, i=strip, m=row-quadrant,
j=unit-in-quadrant; j1=j%2, j0=j//2):
  - top-8 values+indices per unit via DVE max/max_index (u32 indices);
    per-strip flatten DMAs build [128,4] u32 offset tiles (partition
    32m+8j+j6, col i).
  - ONE indirect DMA with multi-offsets builds the W1 block-diagonal lhsT
    (rows of a host-padded w1fp [65536, 128] where row o*1024+f holds
    W1[o,f,:] in column block 32*(o%4)); ONE builds stageX from xT rows
    (row 1024 = zeros kills the j6=6,7 padding slots).
  - L1: per (m,i) one (K=32, M=128, N=256) matmul, 4 m-quadrants issued
    back-to-back on distinct PE row-groups (tile_position=(32m,0)).
  - L2/L3: block-diagonal (K=128, M=128, N=256) per quadrant t=4m+i.
  - L4: (K=128, M=4, N=256) per t on PE col-group m; Sign activation
    (W3 pre-scaled by |W4|, W4 reduced to signs) = the binarize.
  - PE is pre-warmed with dummy matmuls during the topk/gather head so the
    HAM clock gate sits at 2.4 GHz when the real matmuls arrive.
"""

import numpy as np

OUT, IN, HID, B = 512, 1024, 32, 256
NCORES = 8
OSH = OUT // NCORES  # 64 output units per core
WARM_MMS = 22  # PE warm-up matmuls riding the topk/gather head
MULTI_OFF = False  # HW probe: multi-offset indirect only honors partition 0

_CACHE = {}


def _maps():
    # o = 16i + 4m + j ; mask/topk row r = 16i + 4m + j (identity)
    return np.arange(OSH)


def _build_program():
    import concourse.bacc as bacc
    import concourse.bass as bass
    import concourse.mybir as mybir
    import concourse.tile as tile

    f32 = mybir.dt.float32
    u32 = mybir.dt.uint32
    i16 = mybir.dt.int16
    RELU = mybir.ActivationFunctionType.Relu
    SIGN = mybir.ActivationFunctionType.Sign

    nc = bacc.Bacc(None, target_bir_lowering=False, debug=False)

    xT = nc.dram_tensor("xT", [IN + 1, B], f32, kind="ExternalInput")
    maskS = nc.dram_tensor("maskS", [OSH, IN], f32, kind="ExternalInput")
    oconst = nc.dram_tensor("oconst", [OSH, 1], u32, kind="ExternalInput")
    w1fp = nc.dram_tensor("w1fp", [OSH * IN, 4 * HID], f32, kind="ExternalInput")
    w2bd_d = nc.dram_tensor("w2bd", [128, 2048], f32, kind="ExternalInput")
    w3bd_d = nc.dram_tensor("w3bd", [128, 2048], f32, kind="ExternalInput")
    w4sgn = nc.dram_tensor("w4sgn", [128, 64], f32, kind="ExternalInput")
    outS = nc.dram_tensor("outS", [OSH, B], f32, kind="ExternalOutput")
    DBG = _CACHE.get("debug", False)
    if DBG:
        dbg_idx8 = nc.dram_tensor("dbg_idx8", [OSH, 8], u32, kind="ExternalOutput")
        dbg_offx = nc.dram_tensor("dbg_offx", [128, 4], u32, kind="ExternalOutput")
        dbg_bd = nc.dram_tensor("dbg_bd", [128, 512], f32, kind="ExternalOutput")
        dbg_sx = nc.dram_tensor("dbg_sx", [128, 1024], f32, kind="ExternalOutput")
        dbg_h1s = nc.dram_tensor("dbg_h1s", [128, 4096], f32, kind="ExternalOutput")

    with tile.TileContext(nc) as tc:
        with (
            tc.tile_pool(name="const", bufs=1) as cpool,
            tc.tile_pool(name="psw", bufs=1, space="PSUM") as pswarm,
            tc.tile_pool(name="ps", bufs=5, space="PSUM") as pspool,
            tc.tile_pool(name="psy", bufs=1, space="PSUM") as psypool,
        ):
            # --- PE warm-up: dummy matmuls with no upstream deps ---
            warm = cpool.tile([128, 256], f32)
            nc.vector.memset(warm[:], 0.0)
            psw = pswarm.tile([128, 256], f32)
            for k in range(WARM_MMS):
                nc.tensor.matmul(
                    out=psw[:, 0:256],
                    lhsT=warm[:, 0:128],
                    rhs=warm[:, 0:256],
                    start=True,
                    stop=True,
                    tile_position=(0, 0),
                )

            # --- loads ---
            mask_t = cpool.tile([OSH, IN], f32)
            nc.sync.dma_start(mask_t[:], maskS[:])
            oconst_t = cpool.tile([OSH, 1], u32)
            nc.sync.dma_start(oconst_t[:], oconst[:])
            w4t = cpool.tile([128, 64], f32)
            nc.scalar.dma_start(w4t[:], w4sgn[:])
            w2bd = cpool.tile([128, 2048], f32)
            nc.scalar.dma_start(w2bd[:], w2bd_d[:])
            w3bd = cpool.tile([128, 2048], f32)
            nc.scalar.dma_start(w3bd[:], w3bd_d[:])

            # --- top-8 values + indices per unit (6 real, 2 padding) ---
            mx8 = cpool.tile([OSH, 8], f32)
            idx8 = cpool.tile([OSH, 8], u32)
            nc.vector.max(out=mx8[:], in_=mask_t[:])
            nc.vector.max_index(out=idx8[:], in_max=mx8[:], in_values=mask_t[:])

            # gx/gw [64, 8] u32: per-unit-row x-row idx (pads -> zero row
            # IN) and w1fp row idx o*1024 + idx.
            gx = cpool.tile([OSH, 8], u32)
            gw = cpool.tile([OSH, 8], u32)
            nc.vector.tensor_copy(gx[:, 0:6], idx8[:, 0:6])
            nc.vector.memset(gx[:, 6:8], IN)
            nc.vector.tensor_tensor(
                out=gw[:],
                in0=idx8[:],
                in1=oconst_t[:].to_broadcast([OSH, 8]),
                op=mybir.AluOpType.add,
            )

            # offset tiles [128, 4]: off[32m+8j+j6, i] <- g_[16i+4m+j, j6]
            offx = cpool.tile([128, 4], u32)
            offw = cpool.tile([128, 4], u32)
            for i in range(4):
                xeng = nc.sync if i % 2 == 0 else nc.scalar
                weng = nc.scalar if i % 2 == 0 else nc.sync
                xeng.dma_start(
                    out=offx[:, i : i + 1], in_=gx[16 * i : 16 * i + 16, 0:8]
                )
                weng.dma_start(
                    out=offw[:, i : i + 1], in_=gw[16 * i : 16 * i + 16, 0:8]
                )

            # --- indirect row gathers: W -> bd block-diag lhsT; X -> stageX
            bd = cpool.tile([128, 512], f32)
            stageX = cpool.tile([128, 1024], f32)
            if MULTI_OFF:
                nc.gpsimd.indirect_dma_start(
                    out=stageX[:].rearrange("p (i b) -> p i b", b=B),
                    out_offset=None,
                    in_=xT[:, :],
                    in_offset=bass.IndirectOffsetOnAxis(ap=offx[:, 0:4], axis=0),
                )
                nc.gpsimd.indirect_dma_start(
                    out=bd[:].rearrange("p (i q) -> p i q", q=128),
                    out_offset=None,
                    in_=w1fp[:, :],
                    in_offset=bass.IndirectOffsetOnAxis(ap=offw[:, 0:4], axis=0),
                )
            else:
                def xgather(i):
                    nc.gpsimd.indirect_dma_start(
                        out=stageX[:, B * i : B * i + B],
                        out_offset=None,
                        in_=xT[:, :],
                        in_offset=bass.IndirectOffsetOnAxis(
                            ap=offx[:, i : i + 1], axis=0
                        ),
                    )

                def wg(i):
                    nc.gpsimd.indirect_dma_start(
                        out=bd[:, 128 * i : 128 * i + 128],
                        out_offset=None,
                        in_=w1fp[:, :],
                        in_offset=bass.IndirectOffsetOnAxis(
                            ap=offw[:, i : i + 1], axis=0
                        ),
                    )

                for i in (0, 1):
                    xgather(i)
                    wg(i)
                for i in (2, 3):
                    xgather(i)
                    wg(i)

            def evac(dst, src, k):
                # alternate relu evacuation between Scalar and Vector
                if k % 2 == 0:
                    nc.scalar.activation(out=dst, in_=src, func=RELU)
                else:
                    nc.vector.tensor_scalar_max(dst, src, 0.0)

            # --- L1: per (m, i) one (K=32, M=128, N=256) matmul on PE
            # row-quadrant m; psum tile per (m, i-pair) wave ---
            h1s = cpool.tile([128, 4096], f32)
            nk = 0
            for w in range(2):  # wave = strip pair (0,1) then (2,3)
                for m in range(4):
                    ps1 = pspool.tile([128, 512], f32, tag="ps", name=f"ps1_{w}_{m}")
                    for ih in range(2):
                        i = 2 * w + ih
                        nc.tensor.matmul(
                            out=ps1[:, 256 * ih : 256 * ih + 256],
                            lhsT=bd[32 * m : 32 * m + 32, 128 * i : 128 * i + 128],
                            rhs=stageX[32 * m : 32 * m + 32, B * i : B * i + B],
                            start=True,
                            stop=True,
                            tile_position=(32 * m, 0),
                        )
                    # h1s quadrant t = 4m+i -> cols 256t; (m, wave) -> cols
                    # 1024m + 512w
                    evac(h1s[:, 1024 * m + 512 * w : 1024 * m + 512 * w + 512],
                         ps1[:, :], nk)
                    nk += 1

            # --- L2/L3: block-diag (K=128, M=128, N=256) per quadrant t ---
            h2s = cpool.tile([128, 4096], f32)
            h3s = cpool.tile([128, 4096], f32)
            TP_ORDER = [0, 2, 4, 6, 1, 3, 5, 7]  # wave-A-derived pairs first
            for wt, hin, hout in ((w2bd, h1s, h2s), (w3bd, h2s, h3s)):
                for tp in TP_ORDER:
                    ps2 = pspool.tile([128, 512], f32, tag="ps", name=f"ps_{nk}")
                    for ih in range(2):
                        t = 2 * tp + ih
                        nc.tensor.matmul(
                            out=ps2[:, 256 * ih : 256 * ih + 256],
                            lhsT=wt[:, 128 * t : 128 * t + 128],
                            rhs=hin[:, B * t : B * t + B],
                            start=True,
                            stop=True,
                            tile_position=(0, 0),
                        )
                    evac(hout[:, 512 * tp : 512 * tp + 512], ps2[:, :], nk)
                    nk += 1

            # --- L4 + binarize: psy[32m+j, 256i+b] = logit(o=16i+4m+j, b) ---
            psy = psypool.tile([128, 1024], f32)
            nc.vector.memset(psy[:], 0.0)
            ys = cpool.tile([128, 1024], f32)
            for i in (0, 1, 2, 3):
                for m in range(4):
                    t = 4 * m + i
                    nc.tensor.matmul(
                        out=psy[32 * m : 32 * m + 4, 256 * i : 256 * i + 256],
                        lhsT=w4t[:, 4 * t : 4 * t + 4],
                        rhs=h3s[:, B * t : B * t + B],
                        start=True,
                        stop=True,
                        tile_position=(0, 32 * m),
                    )
            nc.scalar.activation(out=ys[:, 0:512], in_=psy[:, 0:512], func=SIGN)
            nc.scalar.activation(out=ys[:, 512:1024], in_=psy[:, 512:1024], func=SIGN)

            if DBG:
                nc.sync.dma_start(dbg_idx8[:], idx8[:])
                nc.sync.dma_start(dbg_offx[:], offx[:])
                nc.sync.dma_start(dbg_bd[:], bd[:])
                nc.sync.dma_start(dbg_sx[:], stageX[:])
                nc.sync.dma_start(dbg_h1s[:], h1s[:])

            # outS[16i+4m+j, b] = ys[32m+j, 256i+b]; one DMA per quadrant m
            for m in range(4):
                eng = nc.sync if m % 2 == 0 else nc.scalar
                eng.dma_start(
                    out=outS[:].rearrange("(i mm j) b -> mm j i b", mm=4, j=4)[
                        m : m + 1
                    ],
                    in_=ys[32 * m : 32 * m + 4, :].rearrange("j (i b) -> j i b", b=B),
                )

    nc.compile()
    return nc


def _prep_core(c, inputs, mask, W1, W2, W3, W4, o_of_r):
    sl = slice(c * OSH, (c + 1) * OSH)
    mask_c = mask[sl]
    W1c, W2c, W3c, W4c = W1[sl], W2[sl], W3[sl], W4[sl]

    maskS = np.ascontiguousarray(mask_c[o_of_r])
    oconst = (o_of_r.astype(np.uint32) * np.uint32(IN))[:, None]

    # w1fp[o*IN + f, 32*(o%4) + h] = W1c[o, f, h]
    w1fp = np.zeros((OSH, IN, 4, HID), np.float32)
    o = np.arange(OSH)
    w1fp[o, :, o % 4, :] = W1c
    w1fp = w1fp.reshape(OSH * IN, 4 * HID)

    # block-diag L2/L3 weights: col block t=4m+i holds lhsT for quadrant t:
    #   w2bd[32j+h, 128t + 32j+k] = W2c[o(t,j), h, k]
    #   w3bd[32j+k, 128t + 32j+l] = W3c[o,k,l] * |W4c[o,l]|
    w4v = W4c[:, :, 0]  # [64, 32]
    w3p = W3c * np.abs(w4v)[:, None, :]
    w2bd = np.zeros((128, 2048), np.float32)
    w3bd = np.zeros((128, 2048), np.float32)
    sgn = np.sign(w4v).astype(np.float32)
    w4sgn = np.zeros((128, 64), np.float32)
    for o in range(OSH):
        i, m, j = o // 16, (o % 16) // 4, o % 4
        t = 4 * m + i
        w2bd[32 * j : 32 * j + 32, 128 * t + 32 * j : 128 * t + 32 * j + 32] = W2c[o]
        w3bd[32 * j : 32 * j + 32, 128 * t + 32 * j : 128 * t + 32 * j + 32] = w3p[o]
        w4sgn[32 * j : 32 * j + 32, 4 * t + j] = sgn[o]

    return {
        "maskS": maskS.astype(np.float32),
        "oconst": oconst,
        "w1fp": w1fp,
        "w2bd": w2bd,
        "w3bd": w3bd,
        "w4sgn": w4sgn,
    }


def kernel(inputs, mask, W1, W2, W3, W4, _run_kwargs=None):
    from concourse.bass_utils import run_bass_kernel_spmd

    inputs = np.asarray(inputs, np.float32)
    mask = np.asarray(mask, np.float32)
    W1 = np.asarray(W1, np.float32)
    W2 = np.asarray(W2, np.float32)
    W3 = np.asarray(W3, np.float32)
    W4 = np.asarray(W4, np.float32)

    if "nc" not in _CACHE:
        _CACHE["nc"] = _build_program()
    nc = _CACHE["nc"]

    o_of_r = _maps()
    xT = np.zeros((IN + 1, B), np.float32)
    xT[:IN] = inputs.T
    in_maps = []
    for c in range(NCORES):
        m = _prep_core(c, inputs, mask, W1, W2, W3, W4, o_of_r)
        m["xT"] = xT
        in_maps.append(m)

    kw = dict(_run_kwargs or {})
    res = run_bass_kernel_spmd(nc, in_maps, core_ids=list(range(NCORES)), **kw)
    out = np.concatenate([r["outS"].T for r in res.results], axis=1)
    if _run_kwargs is not None:
        _CACHE["last_result"] = res
    return out.astype(np.float32)
